# revision 1
# baseline (speedup 1.0000x reference)
"""DoubleAttention TRN2 Bass kernel.

Full inputs in, full outputs out. Data-parallel over batch: B=32 split as
4 batches per core across 8 NeuronCores; 1x1-conv weights replicated.

Per-batch math (C = Cout = dn = 512, N = H*W = 1024):
  A   = wA @ x + bA            [C, N]
  smB = softmax(wB @ x, n)     (bB drops: softmax shift-invariant)
  smV = softmax(wV @ x, n)     (bV drops)
  G   = A @ smB^T              [C, C]
  Z   = wR @ (G @ smV) + bR    [C, N]

Kernel-side formulation (everything float32r on the PE):
  AT[n,c]   = sum_c' x[c',n] wA^T[c',c]        (transposed conv; no transposes
  EBT[n,d]  = exp(sum_c' x[c',n] wB^T[c',d])    needed for the n-contraction)
  EV[d,n]   = exp(Vm[d,n]); sV[d] = sum_n EV[d,n]   (natural layout)
  sB[d]     = sum_n EBT[n,d]    via ones-matmul + rank-1 transpose matmuls
  GrawT[d,c]= sum_n EBT[n,d] AT[n,c]
  GT[d,c]   = GrawT[d,c]/(sB[d]*sV[d]) + bA[c]/sV[d]   (scale+bias on evac)
  Z0[c,n]   = sum_d GT[d,c] EV[d,n]
  out[o,n]  = sum_c wR^T[c,o] Z0[c,n] + bR[o]   (bias via ACT activation)
"""

import numpy as np

B, C, N = 32, 512, 1024  # batch, channels, spatial (32*32)
H = W = 32
NCORES = 8
BPC = B // NCORES   # batches per core
KT = C // 128       # 4 contraction tiles
NT = N // 128       # 8 n-partition tiles
NS = N // 512       # 2 n free-dim spans

_CACHE = {}


def _build_nc():
    import concourse.bacc as bacc
    import concourse.mybir as mybir
    import concourse.tile as tile

    F32 = mybir.dt.float32
    F32R = mybir.dt.float32r
    AF = mybir.ActivationFunctionType

    nc = bacc.Bacc("TRN2", target_bir_lowering=False, debug=False,
                   num_devices=NCORES)
    x_d = nc.dram_tensor("x", [BPC, C, N], F32R, kind="ExternalInput").ap()
    wat_d = nc.dram_tensor("wat", [C, C], F32R, kind="ExternalInput").ap()
    wbt_d = nc.dram_tensor("wbt", [C, C], F32R, kind="ExternalInput").ap()
    wvt_d = nc.dram_tensor("wvt", [C, C], F32R, kind="ExternalInput").ap()
    wrt_d = nc.dram_tensor("wrt", [C, C], F32R, kind="ExternalInput").ap()
    bab_d = nc.dram_tensor("bab", [128, C], F32, kind="ExternalInput").ap()
    br_d = nc.dram_tensor("br", [128, KT], F32, kind="ExternalInput").ap()
    ones_d = nc.dram_tensor("ones", [128, 128], F32R, kind="ExternalInput").ap()
    o_d = nc.dram_tensor("o", [BPC, C, N], F32, kind="ExternalOutput").ap()

    with tile.TileContext(nc) as tc:
        with tc.tile_pool(name="wp", bufs=1) as wp, \
             tc.tile_pool(name="xp", bufs=2) as xp, \
             tc.tile_pool(name="ip", bufs=1) as ip, \
             tc.tile_pool(name="op", bufs=2) as op_, \
             tc.tile_pool(name="sp", bufs=2) as sp, \
             tc.tile_pool(name="pp", bufs=8, space="PSUM") as pp:

            wat = wp.tile([128, KT, C], F32R, tag="wat")
            wbt = wp.tile([128, KT, C], F32R, tag="wbt")
            wvt = wp.tile([128, KT, C], F32R, tag="wvt")
            wrt = wp.tile([128, KT, C], F32R, tag="wrt")
            xs0 = xp.tile([128, KT, N], F32R, tag="xs")
            ones = wp.tile([128, 128], F32R, tag="ones")
            # Warm the PE HAM clock gate during the DMA head: 4 slow fp32
            # matmuls (4 cyc/row) on a memset tile keep the array busy for
            # the ~3.4us SHORT window and finish before the real stream.
            garb = wp.tile([128, 512], F32, tag="garb")
            nc.gpsimd.memset(garb[:], 1.0)
            psw = pp.tile([128, 512], F32, tag="mm")
            for _ in range(2):
                nc.tensor.matmul(psw[:], garb[:, 0:128], garb[:],
                                 start=True, stop=True)
            # DMA priority order for batch 0: the first PV group needs
            # x[:, :, 0:512] plus wvt. Medium chunks on alternating queues
            # maximize early aggregate bandwidth without flooding the SP
            # sequencer with triggers.
            for k in range(KT):
                nc.sync.dma_start(xs0[:, k, 0:512],
                                  x_d[0, k * 128:(k + 1) * 128, 0:512])
                nc.sync.dma_start(wvt[:, k, :],
                                  wvt_d[k * 128:(k + 1) * 128, :])
            nc.sync.dma_start(xs0[:, :, 512:1024],
                              x_d[0, :, 512:1024].rearrange(
                                  "(k p) n -> p k n", p=128))
            for k in range(KT):
                nc.sync.dma_start(wat[:, k, :],
                                  wat_d[k * 128:(k + 1) * 128, :])
                nc.sync.dma_start(wbt[:, k, :],
                                  wbt_d[k * 128:(k + 1) * 128, :])
            nc.sync.dma_start(wrt[:], wrt_d.rearrange("(k p) c -> p k c",
                                                      p=128))
            nc.sync.dma_start(ones[:], ones_d[:])
            bab = wp.tile([128, C], F32, tag="bab")
            nc.sync.dma_start(bab[:], bab_d[:])
            br = wp.tile([128, KT], F32, tag="br")
            nc.sync.dma_start(br[:], br_d[:])

            for b in range(BPC):
                if b == 0:
                    xs = xs0
                else:
                    xs = xp.tile([128, KT, N], F32R, tag="xs")
                    for h in range(NS):
                        hsl = slice(h * 512, (h + 1) * 512)
                        nc.sync.dma_start(
                            xs[:, :, hsl],
                            x_d[b, :, hsl].rearrange("(k p) n -> p k n",
                                                     p=128))

                at = ip.tile([128, NT, C], F32R, tag="at")
                ebt = ip.tile([128, NT, C], F32R, tag="ebt")
                ev = ip.tile([128, KT, N], F32R, tag="ev")
                gt = ip.tile([128, KT, C], F32R, tag="gt")
                zs = ip.tile([128, KT, N], F32R, tag="zs")
                av = sp.tile([128, KT, NS], F32, tag="av")
                svc = sp.tile([128, KT], F32, tag="svc")
                sbc = sp.tile([128, KT], F32, tag="sbc")
                prod = sp.tile([128, KT], F32, tag="prod")
                rsc = sp.tile([128, KT], F32, tag="rsc")
                rsv = sp.tile([128, KT], F32, tag="rsv")
                sbr = sp.tile([1, C], F32R, tag="sbr")
                ebp = [sp.tile([128, C], F32R, tag=f"ebp{i}",
                               name=f"ebp{i}", bufs=1) for i in range(7)]
                os_ = op_.tile([128, KT, N], F32, tag="os")

                # Phase V: EV[d,n] natural + per-row expsums (h outer so
                # the first groups only need the first half of x)
                for h in range(NS):
                    hsl = slice(h * 512, (h + 1) * 512)
                    for dt in range(KT):
                        dsl = slice(dt * 128, (dt + 1) * 128)
                        psv = pp.tile([128, 512], F32, tag="mm")
                        for k in range(KT):
                            nc.tensor.matmul(psv[:], wvt[:, k, dsl],
                                             xs[:, k, hsl],
                                             start=(k == 0), stop=(k == KT - 1))
                        nc.scalar.activation(ev[:, dt, hsl], psv[:], AF.Exp,
                                             accum_out=av[:, dt, h:h + 1])
                nc.vector.tensor_add(svc[:], av[:, :, 0], av[:, :, 1])
                nc.vector.reciprocal(rsv[:], svc[:])

                # Phase 1: AT[n,c] and EBT[n,d] per n-tile.
                # k-paired order: consecutive matmuls share the same stationary
                # xs chunk (one weight set serves psa and psb).
                with nc.allow_low_precision(
                        reason="fp32r partials match the fp32r pipeline"):
                    for nt in range(NT):
                        nsl = slice(nt * 128, (nt + 1) * 128)
                        psa = pp.tile([128, C], F32, tag="mm")
                        psb = pp.tile([128, C], F32, tag="mm")
                        for k in range(KT):
                            nc.tensor.matmul(psa[:], xs[:, k, nsl],
                                             wat[:, k, :],
                                             start=(k == 0),
                                             stop=(k == KT - 1))
                            nc.tensor.matmul(psb[:], xs[:, k, nsl],
                                             wbt[:, k, :],
                                             start=(k == 0),
                                             stop=(k == KT - 1))
                        if nt == 7:
                            # exp + final tree add first: the sB chain is
                            # the critical path into pss; at-copy can wait
                            nc.scalar.activation(ebt[:, nt, :], psb[:],
                                                 AF.Exp)
                            nc.vector.tensor_add(ebp[6][:], ebp[5][:],
                                                 ebt[:, 7, :])
                            nc.vector.tensor_copy(at[:, nt, :], psa[:])
                            continue
                        nc.vector.tensor_copy(at[:, nt, :], psa[:])
                        nc.scalar.activation(ebt[:, nt, :], psb[:], AF.Exp)
                        # skewed sB partial-sum tree: only the nt=7 add
                        # depends on the last exp, so the pss matmul can
                        # issue almost immediately after P1.
                        if nt == 1:
                            nc.vector.tensor_add(ebp[0][:], ebt[:, 0, :],
                                                 ebt[:, 1, :])
                        elif nt == 3:
                            nc.vector.tensor_add(ebp[1][:], ebt[:, 2, :],
                                                 ebt[:, 3, :])
                            nc.vector.tensor_add(ebp[2][:], ebp[0][:],
                                                 ebp[1][:])
                        elif nt == 5:
                            nc.vector.tensor_add(ebp[3][:], ebt[:, 4, :],
                                                 ebt[:, 5, :])
                        elif nt == 6:
                            nc.vector.tensor_add(ebp[4][:], ebp[3][:],
                                                 ebt[:, 6, :])
                            nc.vector.tensor_add(ebp[5][:], ebp[2][:],
                                                 ebp[4][:])

                # Phase G: GrawT[d,c]; evac folds the 1/(sB*sV) scale and
                # the +bA[c] bias (GT = GrawT*rscale + bA_bcast*rsV). The sB
                # reduction (one ones-matmul over the tree total + K=1
                # row->column transpose matmuls) slots in after the first
                # group so its chain hides under the remaining groups.
                def gt_evac(dt, psg):
                    gta = sp.tile([128, C], F32, tag="gta", name="gta")
                    nc.scalar.mul(gta[:], psg[:], rsc[:, dt:dt + 1])
                    tmpb = sp.tile([128, C], F32, tag="tmpb", name="tmpb")
                    nc.vector.tensor_scalar_mul(tmpb[:], bab[:],
                                                rsv[:, dt:dt + 1])
                    nc.vector.tensor_add(gt[:, dt, :], gta[:], tmpb[:])

                # PE order: psg0, pss, psg1, psc, psg2, psg3 — the 1-lane
                # sbr row-copy (pss -> psc dependency) hides under psg1.
                psgs = []
                for dt in range(KT):
                    dsl = slice(dt * 128, (dt + 1) * 128)
                    psg = pp.tile([128, C], F32, tag="mm")
                    for nt in range(NT):
                        nc.tensor.matmul(psg[:], ebt[:, nt, dsl], at[:, nt, :],
                                         start=(nt == 0), stop=(nt == NT - 1))
                    if dt == 0:
                        psgs.append(psg)
                        pss = pp.tile([128, 512], F32, tag="mm")
                        nc.tensor.matmul(pss[:], ones[:], ebp[6][:],
                                         start=True, stop=True)
                        nc.vector.tensor_copy(sbr[:], pss[0:1, :])
                        continue
                    if dt == 1:
                        psgs.append(psg)
                        psc = pp.tile([128, KT, 2], F32, tag="mm")
                        for dtc in range(KT):
                            nc.tensor.matmul(
                                psc[:, dtc, :],
                                sbr[0:1, dtc * 128:(dtc + 1) * 128],
                                ones[0:1, 0:2], start=True, stop=True)
                        nc.vector.tensor_copy(sbc[:], psc[:, :, 0])
                        nc.vector.tensor_mul(prod[:], sbc[:], svc[:])
                        nc.vector.reciprocal(rsc[:], prod[:])
                        gt_evac(0, psgs[0])
                        gt_evac(1, psgs[1])
                        continue
                    gt_evac(dt, psg)

                # Phase Z: Z0[c,n]
                for ct in range(KT):
                    csl = slice(ct * 128, (ct + 1) * 128)
                    for h in range(NS):
                        hsl = slice(h * 512, (h + 1) * 512)
                        psz = pp.tile([128, 512], F32, tag="mm")
                        for dt in range(KT):
                            nc.tensor.matmul(psz[:], gt[:, dt, csl],
                                             ev[:, dt, hsl],
                                             start=(dt == 0), stop=(dt == KT - 1))
                        nc.vector.tensor_copy(zs[:, ct, hsl], psz[:])

                # Phase R: out[o,n] = wR @ Z + bR
                for ot in range(KT):
                    osl = slice(ot * 128, (ot + 1) * 128)
                    for h in range(NS):
                        hsl = slice(h * 512, (h + 1) * 512)
                        psr = pp.tile([128, 512], F32, tag="mm")
                        for k in range(KT):
                            nc.tensor.matmul(psr[:], wrt[:, k, osl],
                                             zs[:, k, hsl],
                                             start=(k == 0), stop=(k == KT - 1))
                        nc.scalar.activation(os_[:, ot, hsl], psr[:],
                                             AF.Identity, bias=br[:, ot:ot + 1])
                        nc.sync.dma_start(
                            o_d[b, ot * 128:(ot + 1) * 128, h * 512:(h + 1) * 512],
                            os_[:, ot, hsl])
    nc.compile()
    return nc


def _in_maps(x, wA, bA, wB, wV, wR, bR):
    xr = np.ascontiguousarray(x.reshape(B, C, N), dtype=np.float32)
    wat = np.ascontiguousarray(wA.T, dtype=np.float32)
    wbt = np.ascontiguousarray(wB.T, dtype=np.float32)
    wvt = np.ascontiguousarray(wV.T, dtype=np.float32)
    wrt = np.ascontiguousarray(wR.T, dtype=np.float32)
    bab = np.ascontiguousarray(
        np.broadcast_to(bA.reshape(1, C), (128, C)), dtype=np.float32)
    br = np.ascontiguousarray(bR.reshape(KT, 128).T, dtype=np.float32)
    ones = np.ones((128, 128), dtype=np.float32)
    maps = []
    for i in range(NCORES):
        maps.append({
            "x": np.ascontiguousarray(xr[i * BPC:(i + 1) * BPC]),
            "wat": wat, "wbt": wbt, "wvt": wvt, "wrt": wrt,
            "bab": bab, "br": br, "ones": ones,
        })
    return maps


def kernel(x, wA, bA, wB, bB, wV, bV, wR, bR):
    from concourse.bass_utils import run_bass_kernel_spmd
    if "nc" not in _CACHE:
        _CACHE["nc"] = _build_nc()
    nc = _CACHE["nc"]
    maps = _in_maps(x, wA, bA, wB, wV, wR, bR)
    res = run_bass_kernel_spmd(nc, maps, list(range(NCORES)))
    out = np.concatenate([res.results[i]["o"] for i in range(NCORES)], axis=0)
    return out.reshape(B, C, H, W).astype(np.float32)



# revision 4
# speedup vs baseline: 1.2548x; 1.2548x over previous
"""DoubleAttention TRN2 Bass kernel — fp8 DoubleRow edition.

Full inputs in, full outputs out. Data-parallel over batch: B=32 split as
4 batches per core across 8 NeuronCores; 1x1-conv weights replicated.

Per-batch math (C = Cout = dn = 512, N = H*W = 1024):
  A   = wA @ x + bA            [C, N]
  smB = softmax(wB @ x, n)     (bB drops: softmax shift-invariant)
  smV = softmax(wV @ x, n)     (bV drops)
  G   = A @ smB^T              [C, C]
  Z   = wR @ (G @ smV) + bR    [C, N]

The PE runs fp8e4 with perf_mode=DoubleRow (2 fp8 weights/cell, K=256 per
instruction — ~2x instruction count reduction over 1-elem/cell dtypes) for
the three input convs, the G contraction, and the Z contraction. The final
R conv stays fp16: its operand quantization passes straight to the output
absmax, so it gets the extra mantissa bits. Scale bookkeeping:

  x8   = SX*x, w{A,B,V}8 = SW*w^T (host)    SX=16  SW=256  SXW=4096
  at8  = SA/SXW * psa = SA*A^T              SA=32
  ebt8 = exp(psb/SXW + CB) = e^CB * e^Bm    CB=1.5 (centers exp in fp8)
  ev8  = exp(psv/SXW + CB), av = rowsums
  svc  = (av0+av1)/SG                       SG=2^17 (lifts GT=G/sV out of
  rsc  = 1/(sB'*svc), rsv = 1/svc                    fp8 subnormals)
  gt8  = psg*rsc + (SA*bA)*rsv  = SA*SG*e^-CB * (Graw/(sB*sV) + bA/sV)
  zs16 = psz/(SA*SG) = Z0 (true scale, fp16)
  out  = wR^T(fp16) @ zs16 + bR             (bf16 out, host upcasts)
"""

import numpy as np

B, C, N = 32, 512, 1024  # batch, channels, spatial (32*32)
H = W = 32
NCORES = 8
BPC = B // NCORES   # batches per core
KT = C // 128       # 4 contraction tiles
NT = N // 128       # 8 n-partition tiles
NS = N // 512       # 2 n free-dim spans

SX = 16.0
SW = 256.0
SXW = SX * SW
SA = 32.0
SG = float(2 ** 17)
CB = 1.5
SZ = 1.0 / (SA * SG)

_CACHE = {}


def _build_nc():
    import concourse.bacc as bacc
    import concourse.mybir as mybir
    import concourse.tile as tile

    F32 = mybir.dt.float32
    F32R = mybir.dt.float32r
    F8 = mybir.dt.float8e4
    F16 = mybir.dt.float16
    BF16 = mybir.dt.bfloat16
    AF = mybir.ActivationFunctionType
    DR = mybir.MatmulPerfMode.DoubleRow

    nc = bacc.Bacc("TRN2", target_bir_lowering=False, debug=False,
                   num_devices=NCORES)
    x_d = nc.dram_tensor("x", [BPC, C, N], F8, kind="ExternalInput").ap()
    wat_d = nc.dram_tensor("wat", [C, C], F8, kind="ExternalInput").ap()
    wbt_d = nc.dram_tensor("wbt", [C, C], F8, kind="ExternalInput").ap()
    wvt_d = nc.dram_tensor("wvt", [C, C], F8, kind="ExternalInput").ap()
    wrt_d = nc.dram_tensor("wrt", [C, C], F16, kind="ExternalInput").ap()
    bab_d = nc.dram_tensor("bab", [128, C], F32, kind="ExternalInput").ap()
    br_d = nc.dram_tensor("br", [128, KT], F32, kind="ExternalInput").ap()
    ones_d = nc.dram_tensor("ones", [128, 128], F32R, kind="ExternalInput").ap()
    o_d = nc.dram_tensor("o", [BPC, C, N], BF16, kind="ExternalOutput").ap()

    with tile.TileContext(nc) as tc:
        with tc.tile_pool(name="wp", bufs=1) as wp, \
             tc.tile_pool(name="xp", bufs=2) as xp, \
             tc.tile_pool(name="ip", bufs=2) as ip, \
             tc.tile_pool(name="op", bufs=2) as op_, \
             tc.tile_pool(name="sp", bufs=2) as sp, \
             tc.tile_pool(name="pp", bufs=8, space="PSUM") as pp:

            wat = wp.tile([128, KT, C], F8, tag="wat")
            wbt = wp.tile([128, KT, C], F8, tag="wbt")
            wvt = wp.tile([128, KT, C], F8, tag="wvt")
            wrt = wp.tile([128, KT, C], F16, tag="wrt")
            xs0 = xp.tile([128, KT, N], F8, tag="xs")
            ones = wp.tile([128, 128], F32R, tag="ones")
            # Warm the PE HAM clock gate during the DMA head: slow fp32
            # matmuls (4 cyc/row) on a memset tile keep the array busy for
            # the ~3.4us SHORT window and finish before the real stream.
            garb = wp.tile([128, 512], F32, tag="garb")
            nc.gpsimd.memset(garb[:], 1.0)
            cbt = wp.tile([128, 1], F32, tag="cbt")
            nc.gpsimd.memset(cbt[:], CB)
            psw = pp.tile([128, 512], F32, tag="mm")
            for _ in range(2):
                nc.tensor.matmul(psw[:], garb[:, 0:128], garb[:],
                                 start=True, stop=True)
            # DMA priority order for batch 0: the first PV group needs
            # x[:, :, 0:512] plus wvt.
            for k in range(KT):
                nc.sync.dma_start(xs0[:, k, 0:512],
                                  x_d[0, k * 128:(k + 1) * 128, 0:512])
                nc.sync.dma_start(wvt[:, k, :],
                                  wvt_d[k * 128:(k + 1) * 128, :])
            nc.sync.dma_start(xs0[:, :, 512:1024],
                              x_d[0, :, 512:1024].rearrange(
                                  "(k p) n -> p k n", p=128))
            for k in range(KT):
                nc.sync.dma_start(wat[:, k, :],
                                  wat_d[k * 128:(k + 1) * 128, :])
                nc.sync.dma_start(wbt[:, k, :],
                                  wbt_d[k * 128:(k + 1) * 128, :])
            nc.sync.dma_start(wrt[:], wrt_d.rearrange("(k p) c -> p k c",
                                                      p=128))
            nc.sync.dma_start(ones[:], ones_d[:])
            bab = wp.tile([128, C], F32, tag="bab")
            nc.sync.dma_start(bab[:], bab_d[:])
            br = wp.tile([128, KT], F32, tag="br")
            nc.sync.dma_start(br[:], br_d[:])

            for b in range(BPC):
                if b == 0:
                    xs = xs0
                else:
                    xs = xp.tile([128, KT, N], F8, tag="xs")
                    for h in range(NS):
                        hsl = slice(h * 512, (h + 1) * 512)
                        nc.sync.dma_start(
                            xs[:, :, hsl],
                            x_d[b, :, hsl].rearrange("(k p) n -> p k n",
                                                     p=128))

                at = ip.tile([128, NT, C], F8, tag="at")
                ebt = ip.tile([128, NT, C], F8, tag="ebt")
                ev = ip.tile([128, KT, N], F8, tag="ev")
                gt = ip.tile([128, KT, C], F8, tag="gt")
                zs = ip.tile([128, KT, N], F16, tag="zs")
                av = sp.tile([128, KT, NS], F32, tag="av")
                svc = sp.tile([128, KT], F32, tag="svc")
                sbc = sp.tile([128, KT], F32, tag="sbc")
                prod = sp.tile([128, KT], F32, tag="prod")
                rsc = sp.tile([128, KT], F32, tag="rsc")
                rsv = sp.tile([128, KT], F32, tag="rsv")
                sbr = sp.tile([1, C], F32R, tag="sbr")
                ebp = [sp.tile([128, C], F32R, tag=f"ebp{i}",
                               name=f"ebp{i}", bufs=1) for i in range(7)]
                os_ = op_.tile([128, KT, N], BF16, tag="os")

                # Phase V: EV[d,n] natural + per-row expsums (h outer so
                # the first groups only need the first half of x)
                for h in range(NS):
                    hsl = slice(h * 512, (h + 1) * 512)
                    for dt in range(KT):
                        dsl = slice(dt * 128, (dt + 1) * 128)
                        psv = pp.tile([128, 512], F32, tag="mm")
                        for k in range(0, KT, 2):
                            nc.tensor.matmul(psv[:], wvt[:, k:k + 2, dsl],
                                             xs[:, k:k + 2, hsl],
                                             start=(k == 0),
                                             stop=(k == KT - 2),
                                             perf_mode=DR)
                        nc.scalar.activation(ev[:, dt, hsl], psv[:], AF.Exp,
                                             bias=cbt[:], scale=1.0 / SXW,
                                             accum_out=av[:, dt, h:h + 1])
                nc.vector.tensor_add(svc[:], av[:, :, 0], av[:, :, 1])
                nc.vector.tensor_scalar_mul(svc[:], svc[:], 1.0 / SG)
                nc.vector.reciprocal(rsv[:], svc[:])

                # Phase 1: AT[n,c] and EBT[n,d] per n-tile.
                # k-paired order: consecutive matmuls share the same
                # stationary xs chunk (one weight set serves psa and psb).
                with nc.allow_low_precision(
                        reason="fp8 pipeline; tolerance is 2e-2"):
                    for nt in range(NT):
                        nsl = slice(nt * 128, (nt + 1) * 128)
                        psa = pp.tile([128, C], F32, tag="mm")
                        psb = pp.tile([128, C], F32, tag="mm")
                        for k in range(0, KT, 2):
                            nc.tensor.matmul(psa[:], xs[:, k:k + 2, nsl],
                                             wat[:, k:k + 2, :],
                                             start=(k == 0),
                                             stop=(k == KT - 2),
                                             perf_mode=DR)
                            nc.tensor.matmul(psb[:], xs[:, k:k + 2, nsl],
                                             wbt[:, k:k + 2, :],
                                             start=(k == 0),
                                             stop=(k == KT - 2),
                                             perf_mode=DR)
                        if nt == 7:
                            # exp + final tree add first: the sB chain is
                            # the critical path into pss; at-copy can wait
                            nc.scalar.activation(ebt[:, nt, :], psb[:],
                                                 AF.Exp, bias=cbt[:],
                                                 scale=1.0 / SXW)
                            nc.vector.tensor_add(ebp[6][:], ebp[5][:],
                                                 ebt[:, 7, :])
                            nc.vector.tensor_scalar_mul(at[:, nt, :], psa[:],
                                                        SA / SXW)
                            continue
                        nc.vector.tensor_scalar_mul(at[:, nt, :], psa[:],
                                                    SA / SXW)
                        nc.scalar.activation(ebt[:, nt, :], psb[:], AF.Exp,
                                             bias=cbt[:], scale=1.0 / SXW)
                        # skewed sB partial-sum tree: only the nt=7 add
                        # depends on the last exp, so the pss matmul can
                        # issue almost immediately after P1.
                        if nt == 1:
                            nc.vector.tensor_add(ebp[0][:], ebt[:, 0, :],
                                                 ebt[:, 1, :])
                        elif nt == 3:
                            nc.vector.tensor_add(ebp[1][:], ebt[:, 2, :],
                                                 ebt[:, 3, :])
                            nc.vector.tensor_add(ebp[2][:], ebp[0][:],
                                                 ebp[1][:])
                        elif nt == 5:
                            nc.vector.tensor_add(ebp[3][:], ebt[:, 4, :],
                                                 ebt[:, 5, :])
                        elif nt == 6:
                            nc.vector.tensor_add(ebp[4][:], ebp[3][:],
                                                 ebt[:, 6, :])
                            nc.vector.tensor_add(ebp[5][:], ebp[2][:],
                                                 ebp[4][:])

                # Phase G: GrawT[d,c]; evac folds the SG/(sB'*sV') scale and
                # the +SA*bA*rsv bias. The sB reduction (ones-matmul over the
                # tree total + K=1 row->column transpose matmuls) slots in
                # after the first group so its chain hides under the rest.
                def gt_evac(dt, psg):
                    gta = sp.tile([128, C], F32, tag="gta", name="gta")
                    nc.scalar.mul(gta[:], psg[:], rsc[:, dt:dt + 1])
                    tmpb = sp.tile([128, C], F32, tag="tmpb", name="tmpb")
                    nc.vector.tensor_scalar_mul(tmpb[:], bab[:],
                                                rsv[:, dt:dt + 1])
                    nc.vector.tensor_add(gt[:, dt, :], gta[:], tmpb[:])

                # PE order: psg0, pss, psg1, psc, psg2, psg3 — the 1-lane
                # sbr row-copy (pss -> psc dependency) hides under psg1.
                psgs = []
                for dt in range(KT):
                    dsl = slice(dt * 128, (dt + 1) * 128)
                    psg = pp.tile([128, C], F32, tag="mm")
                    for nt in range(0, NT, 2):
                        nc.tensor.matmul(psg[:], ebt[:, nt:nt + 2, dsl],
                                         at[:, nt:nt + 2, :],
                                         start=(nt == 0),
                                         stop=(nt == NT - 2),
                                         perf_mode=DR)
                    if dt == 0:
                        psgs.append(psg)
                        pss = pp.tile([128, 512], F32, tag="mm")
                        nc.tensor.matmul(pss[:], ones[:], ebp[6][:],
                                         start=True, stop=True)
                        nc.vector.tensor_copy(sbr[:], pss[0:1, :])
                        continue
                    if dt == 1:
                        psgs.append(psg)
                        psc = pp.tile([128, KT, 2], F32, tag="mm")
                        for dtc in range(KT):
                            nc.tensor.matmul(
                                psc[:, dtc, :],
                                sbr[0:1, dtc * 128:(dtc + 1) * 128],
                                ones[0:1, 0:2], start=True, stop=True)
                        nc.vector.tensor_copy(sbc[:], psc[:, :, 0])
                        nc.vector.tensor_mul(prod[:], sbc[:], svc[:])
                        nc.vector.reciprocal(rsc[:], prod[:])
                        gt_evac(0, psgs[0])
                        gt_evac(1, psgs[1])
                        continue
                    gt_evac(dt, psg)

                # Phase Z: Z0[c,n] (true scale after SZ evac)
                for ct in range(KT):
                    csl = slice(ct * 128, (ct + 1) * 128)
                    for h in range(NS):
                        hsl = slice(h * 512, (h + 1) * 512)
                        psz = pp.tile([128, 512], F32, tag="mm")
                        for dt in range(0, KT, 2):
                            nc.tensor.matmul(psz[:], gt[:, dt:dt + 2, csl],
                                             ev[:, dt:dt + 2, hsl],
                                             start=(dt == 0),
                                             stop=(dt == KT - 2),
                                             perf_mode=DR)
                        nc.vector.tensor_scalar_mul(zs[:, ct, hsl], psz[:],
                                                    SZ)

                # Phase R: out[o,n] = wR @ Z + bR (fp16 matmul)
                for ot in range(KT):
                    osl = slice(ot * 128, (ot + 1) * 128)
                    for h in range(NS):
                        hsl = slice(h * 512, (h + 1) * 512)
                        psr = pp.tile([128, 512], F32, tag="mm")
                        for k in range(KT):
                            nc.tensor.matmul(psr[:], wrt[:, k, osl],
                                             zs[:, k, hsl],
                                             start=(k == 0),
                                             stop=(k == KT - 1))
                        nc.scalar.activation(os_[:, ot, hsl], psr[:],
                                             AF.Identity, bias=br[:, ot:ot + 1])
                        nc.sync.dma_start(
                            o_d[b, ot * 128:(ot + 1) * 128, h * 512:(h + 1) * 512],
                            os_[:, ot, hsl])
    nc.compile()
    return nc


def _in_maps(x, wA, bA, wB, wV, wR, bR):
    import ml_dtypes
    F8NP = ml_dtypes.float8_e4m3
    F16NP = np.float16

    def f8(a):
        return np.ascontiguousarray(
            np.clip(np.asarray(a, np.float32), -240.0, 240.0).astype(F8NP))

    xr = f8(x.reshape(B, C, N) * SX)
    wat = f8(wA.T * SW)
    wbt = f8(wB.T * SW)
    wvt = f8(wV.T * SW)
    wrt = np.ascontiguousarray(wR.T.astype(F16NP))
    bab = np.ascontiguousarray(
        np.broadcast_to((bA * SA).reshape(1, C), (128, C)), dtype=np.float32)
    br = np.ascontiguousarray(bR.reshape(KT, 128).T, dtype=np.float32)
    ones = np.ones((128, 128), dtype=np.float32)
    maps = []
    for i in range(NCORES):
        maps.append({
            "x": np.ascontiguousarray(xr[i * BPC:(i + 1) * BPC]),
            "wat": wat, "wbt": wbt, "wvt": wvt, "wrt": wrt,
            "bab": bab, "br": br, "ones": ones,
        })
    return maps


def kernel(x, wA, bA, wB, bB, wV, bV, wR, bR):
    from concourse.bass_utils import run_bass_kernel_spmd
    if "nc" not in _CACHE:
        _CACHE["nc"] = _build_nc()
    nc = _CACHE["nc"]
    maps = _in_maps(x, wA, bA, wB, wV, wR, bR)
    res = run_bass_kernel_spmd(nc, maps, list(range(NCORES)))
    out = np.concatenate([res.results[i]["o"] for i in range(NCORES)], axis=0)
    return out.reshape(B, C, H, W).astype(np.float32)


# revision 12
# speedup vs baseline: 1.4574x; 1.1615x over previous
"""DoubleAttention TRN2 Bass kernel — fp8 DoubleRow edition.

Full inputs in, full outputs out. Data-parallel over batch: B=32 split as
4 batches per core across 8 NeuronCores; 1x1-conv weights replicated.

Per-batch math (C = Cout = dn = 512, N = H*W = 1024):
  A   = wA @ x + bA            [C, N]
  smB = softmax(wB @ x, n)     (bB drops: softmax shift-invariant)
  smV = softmax(wV @ x, n)     (bV drops)
  G   = A @ smB^T              [C, C]
  Z   = wR @ (G @ smV) + bR    [C, N]

The PE runs fp8e4 with perf_mode=DoubleRow (2 fp8 weights/cell, K=256 per
instruction — ~2x instruction count reduction over 1-elem/cell dtypes) for
the three input convs, the G contraction, and the Z contraction. The final
R conv stays fp16: its operand quantization passes straight to the output
absmax, so it gets the extra mantissa bits. Scale bookkeeping:

  x8   = SX*x, w{A,B,V}8 = SW*w^T (host)    SX=16  SW=256  SXW=4096
  at8  = SA/SXW * psa = SA*A^T              SA=32
  ebt8 = exp(psb/SXW + CB) = e^CB * e^Bm    CB=1.5 (centers exp in fp8)
  ev8  = exp(psv/SXW + CB), av = rowsums
  svc  = (av0+av1)/SG                       SG=2^17 (lifts GT=G/sV out of
  rsc  = 1/(sB'*svc), rsv = 1/svc                    fp8 subnormals)
  gt8  = psg*rsc + (SA*bA)*rsv  = SA*SG*e^-CB * (Graw/(sB*sV) + bA/sV)
  zs16 = psz/(SA*SG) = Z0 (true scale, fp16)
  out  = wR^T(fp16) @ zs16 + bR             (bf16 out, host upcasts)
"""

import numpy as np

B, C, N = 32, 512, 1024  # batch, channels, spatial (32*32)
H = W = 32
NCORES = 8
BPC = B // NCORES   # batches per core
KT = C // 128       # 4 contraction tiles
NT = N // 128       # 8 n-partition tiles
NS = N // 512       # 2 n free-dim spans

SX = 16.0
SW = 256.0
SXW = SX * SW
SA = 32.0
SG = float(2 ** 17)
CB = 1.5
SZ = 1.0 / (SA * SG)

_CACHE = {}


def _build_nc():
    import concourse.bacc as bacc
    import concourse.mybir as mybir
    import concourse.tile as tile

    F32 = mybir.dt.float32
    F32R = mybir.dt.float32r
    F8 = mybir.dt.float8e4
    F16 = mybir.dt.float16
    BF16 = mybir.dt.bfloat16
    AF = mybir.ActivationFunctionType
    DR = mybir.MatmulPerfMode.DoubleRow

    nc = bacc.Bacc("TRN2", target_bir_lowering=False, debug=False,
                   num_devices=NCORES)
    x_d = nc.dram_tensor("x", [BPC, C, N], F8, kind="ExternalInput").ap()
    wat_d = nc.dram_tensor("wat", [C, C], F8, kind="ExternalInput").ap()
    wbt_d = nc.dram_tensor("wbt", [C, C], F8, kind="ExternalInput").ap()
    wvt_d = nc.dram_tensor("wvt", [C, C], F8, kind="ExternalInput").ap()
    wrt_d = nc.dram_tensor("wrt", [C, C], F16, kind="ExternalInput").ap()
    bab_d = nc.dram_tensor("bab", [128, C], F32, kind="ExternalInput").ap()
    br_d = nc.dram_tensor("br", [128, KT], F32, kind="ExternalInput").ap()
    ones_d = nc.dram_tensor("ones", [128, 128], F32R, kind="ExternalInput").ap()
    o_d = nc.dram_tensor("o", [BPC, C, N], BF16, kind="ExternalOutput").ap()

    with tile.TileContext(nc) as tc:
        with tc.tile_pool(name="wp", bufs=1) as wp, \
             tc.tile_pool(name="xp", bufs=2) as xp, \
             tc.tile_pool(name="ip", bufs=2) as ip, \
             tc.tile_pool(name="op", bufs=2) as op_, \
             tc.tile_pool(name="sp", bufs=2) as sp, \
             tc.tile_pool(name="pp", bufs=8, space="PSUM") as pp:

            wat = wp.tile([128, KT, C], F8, tag="wat")
            wbt = wp.tile([128, KT, C], F8, tag="wbt")
            wvt = wp.tile([128, KT, C], F8, tag="wvt")
            wrt = wp.tile([128, KT, C], F16, tag="wrt")
            xs0 = xp.tile([128, KT, N], F8, tag="xs")
            ones = wp.tile([128, 128], F32R, tag="ones")
            # Warm the PE HAM clock gate during the DMA head: slow fp32
            # matmuls (4 cyc/row) on a memset tile keep the array busy for
            # the ~3.4us SHORT window and finish before the real stream.
            garb = wp.tile([128, 512], F32, tag="garb")
            nc.gpsimd.memset(garb[:], 1.0)
            cbt = wp.tile([128, 1], F32, tag="cbt")
            nc.gpsimd.memset(cbt[:], CB)
            # fp8 ones column pair for the DoubleRow sB reduction; middle
            # stride 16 elements keeps the DR weight AP step%16==0 legal.
            ones8 = wp.tile([128, 2, 16], F8, tag="ones8")
            nc.gpsimd.memset(ones8[:], 1.0)
            psw = pp.tile([128, 512], F32, tag="mm")
            for _ in range(2):
                nc.tensor.matmul(psw[:], garb[:, 0:128], garb[:],
                                 start=True, stop=True)
            # DMA priority order for batch 0: the first PV group needs
            # x[:, :, 0:512] plus wvt.
            for k in range(KT):
                nc.sync.dma_start(xs0[:, k, 0:512],
                                  x_d[0, k * 128:(k + 1) * 128, 0:512])
                nc.sync.dma_start(wvt[:, k, :],
                                  wvt_d[k * 128:(k + 1) * 128, :])
            nc.sync.dma_start(xs0[:, :, 512:1024],
                              x_d[0, :, 512:1024].rearrange(
                                  "(k p) n -> p k n", p=128))
            for k in range(KT):
                nc.sync.dma_start(wat[:, k, :],
                                  wat_d[k * 128:(k + 1) * 128, :])
                nc.sync.dma_start(wbt[:, k, :],
                                  wbt_d[k * 128:(k + 1) * 128, :])
            nc.sync.dma_start(wrt[:], wrt_d.rearrange("(k p) c -> p k c",
                                                      p=128))
            nc.sync.dma_start(ones[:], ones_d[:])
            bab = wp.tile([128, C], F32, tag="bab")
            nc.sync.dma_start(bab[:], bab_d[:])
            br = wp.tile([128, KT], F32, tag="br")
            nc.sync.dma_start(br[:], br_d[:])

            for b in range(BPC):
                if b == 0:
                    xs = xs0
                else:
                    xs = xp.tile([128, KT, N], F8, tag="xs")
                    for h in range(NS):
                        hsl = slice(h * 512, (h + 1) * 512)
                        nc.sync.dma_start(
                            xs[:, :, hsl],
                            x_d[b, :, hsl].rearrange("(k p) n -> p k n",
                                                     p=128))

                at = ip.tile([128, NT, C], F8, tag="at")
                ebt = ip.tile([128, NT, C], F8, tag="ebt")
                ev = ip.tile([128, KT, N], F8, tag="ev")
                gt = ip.tile([128, KT, C], F8, tag="gt")
                zs = ip.tile([128, KT, N], F16, tag="zs")
                av = sp.tile([128, KT, NS], F32, tag="av")
                svc = sp.tile([128, KT], F32, tag="svc")
                sbc = sp.tile([128, KT], F32, tag="sbc")
                prod = sp.tile([128, KT], F32, tag="prod")
                rsc = sp.tile([128, KT], F32, tag="rsc")
                rsv = sp.tile([128, KT], F32, tag="rsv")
                sbr = sp.tile([1, C], F32R, tag="sbr")
                os_ = op_.tile([128, KT, N], BF16, tag="os")

                # Phase V: EV[d,n] natural + per-row expsums (h outer so
                # the first groups only need the first half of x)
                for h in range(NS):
                    hsl = slice(h * 512, (h + 1) * 512)
                    for dt in range(KT):
                        dsl = slice(dt * 128, (dt + 1) * 128)
                        psv = pp.tile([128, 512], F32, tag="mm")
                        for k in range(0, KT, 2):
                            nc.tensor.matmul(psv[:], wvt[:, k:k + 2, dsl],
                                             xs[:, k:k + 2, hsl],
                                             start=(k == 0),
                                             stop=(k == KT - 2),
                                             perf_mode=DR)
                        nc.scalar.activation(ev[:, dt, hsl], psv[:], AF.Exp,
                                             bias=cbt[:], scale=1.0 / SXW,
                                             accum_out=av[:, dt, h:h + 1])
                with tc.high_priority():
                    nc.vector.tensor_add(svc[:], av[:, :, 0], av[:, :, 1])
                    nc.vector.tensor_scalar_mul(svc[:], svc[:], 1.0 / SG)
                    nc.vector.reciprocal(rsv[:], svc[:])

                # Phase 1: AT[n,c] and EBT[n,d] per n-tile.
                # k-paired order: consecutive matmuls share the same
                # stationary xs chunk (one weight set serves psa and psb).
                with nc.allow_low_precision(
                        reason="fp8 pipeline; tolerance is 2e-2"):
                    for nt in range(NT):
                        nsl = slice(nt * 128, (nt + 1) * 128)
                        psa = pp.tile([128, C], F32, tag="mm")
                        psb = pp.tile([128, C], F32, tag="mm")
                        for k in range(0, KT, 2):
                            nc.tensor.matmul(psa[:], xs[:, k:k + 2, nsl],
                                             wat[:, k:k + 2, :],
                                             start=(k == 0),
                                             stop=(k == KT - 2),
                                             perf_mode=DR)
                            nc.tensor.matmul(psb[:], xs[:, k:k + 2, nsl],
                                             wbt[:, k:k + 2, :],
                                             start=(k == 0),
                                             stop=(k == KT - 2),
                                             perf_mode=DR)
                        nc.vector.tensor_scalar_mul(at[:, nt, :], psa[:],
                                                    SA / SXW)
                        nc.scalar.activation(ebt[:, nt, :], psb[:], AF.Exp,
                                             bias=cbt[:], scale=1.0 / SXW)

                # Phase G: GrawT[d,c]; evac folds the SG/(sB'*sV') scale and
                # the +SA*bA*rsv bias. The sB reduction (ones-matmul over the
                # tree total + K=1 row->column transpose matmuls) slots in
                # after the first group so its chain hides under the rest.
                def gt_evac(dt, psg):
                    with tc.high_priority():
                        gta = sp.tile([128, C], F32, tag="gta", name="gta")
                        nc.scalar.mul(gta[:], psg[:], rsc[:, dt:dt + 1])
                        tmpb = sp.tile([128, C], F32, tag="tmpb", name="tmpb")
                        nc.vector.tensor_scalar_mul(tmpb[:], bab[:],
                                                    rsv[:, dt:dt + 1])
                        nc.vector.tensor_add(gt[:, dt, :], gta[:], tmpb[:])

                # PE order: psg0, pss, psg1, psc, psg2, psg3 — the sB
                # reduction runs on the PE (ones-column DR matmuls over
                # ebt), keeping the congested DVE off the rsc chain; the
                # 1-lane sbr row-copy hides under psg1.
                psgs = []
                for dt in range(KT):
                    dsl = slice(dt * 128, (dt + 1) * 128)
                    psg = pp.tile([128, C], F32, tag="mm")
                    for nt in range(0, NT, 2):
                        nc.tensor.matmul(psg[:], ebt[:, nt:nt + 2, dsl],
                                         at[:, nt:nt + 2, :],
                                         start=(nt == 0),
                                         stop=(nt == NT - 2),
                                         perf_mode=DR)
                    if dt == 0:
                        psgs.append(psg)
                        pss = pp.tile([128, 512], F32, tag="mm")
                        for nt in range(0, NT, 2):
                            nc.tensor.matmul(pss[0:1, :],
                                             ones8[:, :, 0:1],
                                             ebt[:, nt:nt + 2, :],
                                             start=(nt == 0),
                                             stop=(nt == NT - 2),
                                             perf_mode=DR)
                        with tc.high_priority():
                            nc.vector.tensor_copy(sbr[:], pss[0:1, :])
                        continue
                    if dt == 1:
                        psgs.append(psg)
                        psc = pp.tile([128, KT, 2], F32, tag="mm")
                        for dtc in range(KT):
                            nc.tensor.matmul(
                                psc[:, dtc, :],
                                sbr[0:1, dtc * 128:(dtc + 1) * 128],
                                ones[0:1, 0:2], start=True, stop=True)
                        with tc.high_priority():
                            nc.vector.tensor_copy(sbc[:], psc[:, :, 0])
                            nc.vector.tensor_mul(prod[:], sbc[:], svc[:])
                            nc.vector.reciprocal(rsc[:], prod[:])
                        gt_evac(0, psgs[0])
                        gt_evac(1, psgs[1])
                        continue
                    gt_evac(dt, psg)

                # Phase Z: Z0[c,n] (true scale after SZ evac)
                for ct in range(KT):
                    csl = slice(ct * 128, (ct + 1) * 128)
                    for h in range(NS):
                        hsl = slice(h * 512, (h + 1) * 512)
                        psz = pp.tile([128, 512], F32, tag="mm")
                        for dt in range(0, KT, 2):
                            nc.tensor.matmul(psz[:], gt[:, dt:dt + 2, csl],
                                             ev[:, dt:dt + 2, hsl],
                                             start=(dt == 0),
                                             stop=(dt == KT - 2),
                                             perf_mode=DR)
                        nc.vector.tensor_scalar_mul(zs[:, ct, hsl], psz[:],
                                                    SZ)

                # Phase R: out[o,n] = wR @ Z + bR (fp16 matmul)
                for ot in range(KT):
                    osl = slice(ot * 128, (ot + 1) * 128)
                    for h in range(NS):
                        hsl = slice(h * 512, (h + 1) * 512)
                        psr = pp.tile([128, 512], F32, tag="mm")
                        for k in range(KT):
                            nc.tensor.matmul(psr[:], wrt[:, k, osl],
                                             zs[:, k, hsl],
                                             start=(k == 0),
                                             stop=(k == KT - 1))
                        nc.scalar.activation(os_[:, ot, hsl], psr[:],
                                             AF.Identity, bias=br[:, ot:ot + 1])
                        nc.sync.dma_start(
                            o_d[b, ot * 128:(ot + 1) * 128, h * 512:(h + 1) * 512],
                            os_[:, ot, hsl])
    nc.compile()
    return nc


def _in_maps(x, wA, bA, wB, wV, wR, bR):
    import ml_dtypes
    F8NP = ml_dtypes.float8_e4m3
    F16NP = np.float16

    def f8(a):
        return np.ascontiguousarray(
            np.clip(np.asarray(a, np.float32), -240.0, 240.0).astype(F8NP))

    xr = f8(x.reshape(B, C, N) * SX)
    wat = f8(wA.T * SW)
    wbt = f8(wB.T * SW)
    wvt = f8(wV.T * SW)
    wrt = np.ascontiguousarray(wR.T.astype(F16NP))
    bab = np.ascontiguousarray(
        np.broadcast_to((bA * SA).reshape(1, C), (128, C)), dtype=np.float32)
    br = np.ascontiguousarray(bR.reshape(KT, 128).T, dtype=np.float32)
    ones = np.ones((128, 128), dtype=np.float32)
    maps = []
    for i in range(NCORES):
        maps.append({
            "x": np.ascontiguousarray(xr[i * BPC:(i + 1) * BPC]),
            "wat": wat, "wbt": wbt, "wvt": wvt, "wrt": wrt,
            "bab": bab, "br": br, "ones": ones,
        })
    return maps


def kernel(x, wA, bA, wB, bB, wV, bV, wR, bR):
    from concourse.bass_utils import run_bass_kernel_spmd
    if "nc" not in _CACHE:
        _CACHE["nc"] = _build_nc()
    nc = _CACHE["nc"]
    maps = _in_maps(x, wA, bA, wB, wV, wR, bR)
    res = run_bass_kernel_spmd(nc, maps, list(range(NCORES)))
    out = np.concatenate([res.results[i]["o"] for i in range(NCORES)], axis=0)
    return out.reshape(B, C, H, W).astype(np.float32)
